# revision 8
# baseline (speedup 1.0000x reference)
"""CRDLoss_v2 Trainium2 kernel (8 NeuronCores).

Device: per core, compute S_b = sum_r mask[b,r] * exp(dot(bank_r, f_s_b)/T - SHIFT)
via PE matmul (f_sT stationary, bankT moving) + ACT exp + DVE masked reduce.
Host: exact f64 refinement of the heavy-tail candidate batch rows, analytic
loss assembly, and the trivial 256-row momentum scatter for new_memory.
"""
import sys, os
sys.path.insert(0, "/opt/trn_rl_repo")
import numpy as np
import ml_dtypes

EPS = 1e-07
N_DATA = 500000
NCE_T = 0.07
NCE_M = 0.5
B, D, K = 256, 128, 4096
NCORES = 8
SHARD = N_DATA // NCORES            # 62500
CH = 512                            # matmul moving chunk (one PSUM bank)
GRP = 4                             # chunks per group
GCH = CH * GRP                      # 2048 columns per group
RPAD = ((SHARD + GCH - 1) // GCH) * GCH  # 63488

_BF16 = ml_dtypes.bfloat16
_prog_cache = {}


def build_program(rpad=RPAD, repeat=1):
    import concourse.bacc as bacc
    import concourse.mybir as mybir
    import concourse.tile as tile
    from contextlib import ExitStack

    key = (rpad, repeat)
    if key in _prog_cache:
        return _prog_cache[key]

    ngr = rpad // GCH
    nc = bacc.Bacc("TRN2", debug=False)
    bkT = nc.dram_tensor("bkT", [128, rpad], mybir.dt.bfloat16, kind="ExternalInput")
    fsT = nc.dram_tensor("fsT", [128, 256], mybir.dt.bfloat16, kind="ExternalInput")
    msk = nc.dram_tensor("msk", [256, rpad], mybir.dt.bfloat16, kind="ExternalInput")
    shiftv = nc.dram_tensor("shiftv", [128, 1], mybir.dt.float32, kind="ExternalInput")
    acc = nc.dram_tensor("acc", [128, 2], mybir.dt.float32, kind="ExternalOutput")

    with tile.TileContext(nc) as tc:
        with (
            tc.tile_pool(name="fix", bufs=1) as fix,
            tc.tile_pool(name="st", bufs=3) as st,
            tc.tile_pool(name="ps", bufs=2, space="PSUM") as ps_pool,
        ):
            fst = fix.tile([128, 256], mybir.dt.bfloat16)
            nc.sync.dma_start(fst[:], fsT[:])
            sh = fix.tile([128, 1], mybir.dt.float32)
            nc.sync.dma_start(sh[:], shiftv[:])
            racc = fix.tile([128, 2 * ngr], mybir.dt.float32)
            acct = fix.tile([128, 2], mybir.dt.float32)

            for g in range(ngr * repeat):
                g = g % ngr
                bt = st.tile([128, GCH], mybir.dt.bfloat16, tag="bt")
                nc.sync.dma_start(bt[:], bkT[:, g * GCH:(g + 1) * GCH])
                for h in range(2):
                    mt = st.tile([128, GCH], mybir.dt.bfloat16, tag="mt")
                    nc.sync.dma_start(mt[:], msk[h * 128:(h + 1) * 128, g * GCH:(g + 1) * GCH])
                    pst = ps_pool.tile([128, GCH], mybir.dt.float32, tag="ps")
                    et = st.tile([128, GCH], mybir.dt.bfloat16, tag="et")
                    for k in range(GRP):
                        nc.tensor.matmul(pst[:, k * CH:(k + 1) * CH],
                                         fst[:, h * 128:(h + 1) * 128],
                                         bt[:, k * CH:(k + 1) * CH],
                                         start=True, stop=True)
                        nc.scalar.activation(et[:, k * CH:(k + 1) * CH],
                                             pst[:, k * CH:(k + 1) * CH],
                                             mybir.ActivationFunctionType.Exp,
                                             bias=sh[:, :], scale=float(1.0 / NCE_T))
                    prod = st.tile([128, GCH], mybir.dt.bfloat16, tag="prod")
                    nc.vector.tensor_tensor(out=prod[:], in0=et[:], in1=mt[:],
                                            op=mybir.AluOpType.mult)
                    nc.vector.tensor_reduce(out=racc[:, h * ngr + g:h * ngr + g + 1],
                                            in_=prod[:], axis=mybir.AxisListType.X,
                                            op=mybir.AluOpType.add)
            for h in range(2):
                nc.vector.tensor_reduce(
                    out=acct[:, h:h + 1],
                    in_=racc[:, h * ngr:(h + 1) * ngr],
                    axis=mybir.AxisListType.X, op=mybir.AluOpType.add)
            nc.sync.dma_start(acc[:], acct[:])
    nc.compile()
    _prog_cache[key] = nc
    return nc


def _l2n(x):
    return x / np.sqrt((x * x).sum(1, keepdims=True))


def _host_loss_fallback(f_s, memory_t, all_idx):
    """Full-precision host computation (safety net)."""
    fs = _l2n(f_s.astype(np.float64))
    mem = memory_t.astype(np.float64)
    tot = 0.0
    e_sum = 0.0
    xs = []
    for b in range(B):
        xb = mem[all_idx[b]] @ fs[b] / NCE_T
        xs.append(xb)
        e_sum += np.exp(xb - 50.0).sum() * np.exp(50.0) if xb.max() < 700 else np.inf
    Z = e_sum / (B * (K + 1)) * N_DATA
    mPn = K / N_DATA
    for b in range(B):
        out = np.exp(xs[b]) / Z
        pos, neg = out[0], out[1:]
        tot += np.log(pos / (pos + mPn + EPS))
        tot += np.log(mPn / (neg + mPn + EPS)).sum()
    return -tot / B


def _compute_loss(f_s, memory_t, idx, contrast_idx, S_b, SHIFT):
    fs64 = _l2n(f_s.astype(np.float64))
    mem64 = memory_t.astype(np.float64)
    all_idx = np.concatenate([idx[:, None], contrast_idx], 1).astype(np.int64)
    mPn = K / N_DATA
    c2 = mPn + EPS

    x_pos = (mem64[idx.astype(np.int64)] * fs64).sum(1) / NCE_T

    tot_mass = S_b.sum()
    if not np.isfinite(tot_mass) or tot_mass <= 0:
        return _host_loss_fallback(f_s, memory_t, all_idx)

    Zdev = tot_mass * np.exp(SHIFT) / (B * (K + 1)) * N_DATA
    utol = 1e-4
    cand = np.where(S_b * np.exp(SHIFT) / (Zdev * c2) > utol)[0]
    if len(cand) > 128:
        return _host_loss_fallback(f_s, memory_t, all_idx)

    exp_cand = {}
    for b in cand:
        xb = mem64[all_idx[b]] @ fs64[b] / NCE_T
        exp_cand[b] = np.exp(xb)

    noncand = np.setdiff1d(np.arange(B), cand)
    mass_cand = sum(e.sum() for e in exp_cand.values())
    mass_noncand = S_b[noncand].sum() * np.exp(SHIFT)
    Z = (mass_cand + mass_noncand) / (B * (K + 1)) * N_DATA

    total = 0.0
    for b in cand:
        out = exp_cand[b] / Z
        pos, neg = out[0], out[1:]
        total += np.log(pos / (pos + mPn + EPS))
        total += np.log(mPn / (neg + mPn + EPS)).sum()
    if len(noncand):
        ep = np.exp(x_pos[noncand])
        pos = ep / Z
        total += np.log(pos / (pos + mPn + EPS)).sum()
        negmass = np.maximum(S_b[noncand] * np.exp(SHIFT) - ep, 0.0)
        total += len(noncand) * K * np.log(mPn / c2)
        total += -(negmass / (Z * c2)).sum()
    return -total / B


def kernel(f_s, f_t, memory_t, idx, contrast_idx, epoch=None):
    from concourse.bass_utils import run_bass_kernel_spmd

    f_s = np.asarray(f_s, dtype=np.float32)
    f_t = np.asarray(f_t, dtype=np.float32)
    memory_t = np.asarray(memory_t, dtype=np.float32)
    idx_dt = np.asarray(idx).dtype
    idx = np.asarray(idx).astype(np.int64)
    contrast_idx = np.asarray(contrast_idx).astype(np.int64)

    # ---------- host prep ----------
    fs32 = _l2n(f_s)
    nrm2max = float((memory_t.astype(np.float64) ** 2).sum(1).max())
    SHIFT = float(np.sqrt(nrm2max) / NCE_T - 80.0)

    mem_bf = memory_t.astype(_BF16)
    fsT_in = np.ascontiguousarray(fs32.astype(_BF16).T)          # [128, 256]
    shift_in = np.full((128, 1), -SHIFT, np.float32)

    all_b = np.repeat(np.arange(B, dtype=np.int64), K + 1)
    all_r = np.concatenate([idx[:, None], contrast_idx], 1).ravel()

    in_maps = []
    order = np.argsort(all_r, kind="stable")
    r_sorted = all_r[order]
    b_sorted = all_b[order]
    bounds = np.searchsorted(r_sorted, np.arange(0, N_DATA + 1, SHARD))
    for c in range(NCORES):
        lo = c * SHARD
        bkT = np.zeros((128, RPAD), _BF16)
        bkT[:, :SHARD] = mem_bf[lo:lo + SHARD].T
        msk = np.zeros((256, RPAD), _BF16)
        rs = r_sorted[bounds[c]:bounds[c + 1]] - lo
        bs = b_sorted[bounds[c]:bounds[c + 1]]
        msk[bs, rs] = _BF16(1.0)
        in_maps.append({"bkT": bkT, "fsT": fsT_in, "msk": msk, "shiftv": shift_in})

    # ---------- device ----------
    nc = build_program()
    res = run_bass_kernel_spmd(nc, in_maps, core_ids=list(range(NCORES)))
    S_b = np.zeros(B, np.float64)
    for c in range(NCORES):
        a = res.results[c]["acc"].astype(np.float64)             # [128, 2]
        S_b += np.concatenate([a[:, 0], a[:, 1]])

    # ---------- host loss ----------
    loss = _compute_loss(f_s, memory_t, idx, contrast_idx, S_b, SHIFT)

    # ---------- new_memory ----------
    ft32 = _l2n(f_t)
    w_pos = memory_t[idx] * np.float32(NCE_M) + ft32 * np.float32(1.0 - NCE_M)
    w_pos = _l2n(w_pos)
    new_memory = memory_t.copy()
    new_memory[idx] = w_pos

    return np.float32(loss), new_memory
